# revision 56
# baseline (speedup 1.0000x reference)
"""Single-head causal attention (B=4, S=4096, E=768, D=64) on 8 TRN2 NeuronCores.

Sharding: data-parallel over (batch, query-half): core c -> batch c//2, half c%2.
Each core receives its batch's hidden state pre-transposed to [E, S] in bf16
(host-side layout + dtype choice) and computes attention output for 2048 queries.

Per-core schedule (SPMD-uniform): queries are processed in 4 slots of 512.
Slot j covers keys [0, 1024*(j+1)).  For half p=0 the core owns the upper
512-query chunk of each 1024-block, for p=1 the lower one; the host swaps the
two 512-column halves of each 1024-key block of h^T for p=1 so both halves run
the *same* program:
  - keys [0, 1024j)            : fully unmasked
  - keys [1024j, 1024j+512)    : "dead zone" -- past for p=0 (keep), future for
                                 p=1 (killed via per-core exp bias of -80)
  - keys [1024j+512, 1024(j+1)): the core's own 512 queries -> block-triangular
                                 (0/1 multiplicative mask + memsets on exp out)

Projection: one packed [Wk|Wv] matmul per (octave, e-chunk) produces K^T stacked
on V^T in a pair of [128, 512] PSUM banks; DVE applies the stacked bias and
writes the KV tensor (K^T rows 0-63, V^T rows 64-127). V tiles for the PV
matmul come from PE transposes of a base-partition-0 staging copy; the
transposes are spread across the *next* slot's supers so the PE matmul stream
stays dense (HAM clock-gate stays warm). Q (own half only) gets bias + 1/8
scale on DVE.

Attention per slot: supers of 2 key-tiles. The two score matmuls of a super run
*concurrently* in different PE row-groups (contract dim is only 64): tile A
contracts array rows 0-63 (K^T / Q^T at partitions 0-63), tile B rows 64-127
using copies of K^T / Q^T DMA-duplicated to partitions 64-127. One Exp ACTIVATE
per super (per-core dead bias folded in); tri-mask on DVE; PV accumulates
[V|1]^T @ P so row 64 collects softmax denominators. Normalization: DVE
reciprocal_approx_fast on the denominator row, bf16 ones-broadcast matmul, DVE
multiply. Emission order interleaves proj octaves with attention slots.
"""

import numpy as np
import ml_dtypes

import concourse.bass as bass
import concourse.tile as tile
from concourse import bacc, mybir
from concourse import bass_utils
from concourse.masks import make_identity

B, S, E, D = 4, 4096, 768, 64
N_CORES = 8
CHUNK = 512            # queries per slot
N_SLOTS = 4            # slots per core (4 * 512 = 2048 queries)
F32 = mybir.dt.float32
BF16 = mybir.dt.bfloat16
DEAD_BIAS = -80.0
ROW_TILE = True        # run each super's two score matmuls in parallel row-groups
EXP_SPLIT = False      # two [128,512] Exp ACTIVATEs per super instead of one [128,1024]


def _strided_cols(t: bass.AP, start: int, stride: int, count: int, width: int) -> bass.AP:
    """AP selecting `count` column-blocks of `width` at `start`, `start+stride`, ..."""
    base = t[:, start : start + 1]
    return bass.AP(
        tensor=base.tensor,
        offset=base.offset,
        ap=[base.ap[0], [stride, count], [1, width]],
    )


def build(debug_dump=False):
    nc = bacc.Bacc("TRN2", target_bir_lowering=False, debug=False, num_devices=N_CORES)

    ht = nc.dram_tensor("ht", [E, S], BF16, kind="ExternalInput").ap()
    wkv = nc.dram_tensor("wkv", [128, 6, 128], BF16, kind="ExternalInput").ap()
    wq = nc.dram_tensor("wq", [128, 6, D], BF16, kind="ExternalInput").ap()
    bkv = nc.dram_tensor("bkv", [128, 1], F32, kind="ExternalInput").ap()
    bq = nc.dram_tensor("bq", [D, 1], F32, kind="ExternalInput").ap()
    dead = nc.dram_tensor("dead", [128, 1], F32, kind="ExternalInput").ap()
    tri2 = nc.dram_tensor("tri2", [128, 256], BF16, kind="ExternalInput").ap()
    ident_in = nc.dram_tensor("ident", [128, 128], BF16, kind="ExternalInput").ap()
    out = nc.dram_tensor("out", [D, N_SLOTS * CHUNK], F32, kind="ExternalOutput").ap()
    if debug_dump:
        kv_dbg = nc.dram_tensor("kv_dbg", [128, S], BF16, kind="ExternalOutput").ap()
        qt_dbg = nc.dram_tensor("qt_dbg", [D, N_SLOTS * CHUNK], BF16, kind="ExternalOutput").ap()
        vh_dbg = nc.dram_tensor("vh_dbg", [128, (S // 128) * (D + 1)], BF16, kind="ExternalOutput").ap()

    n_oct = S // 1024            # 4 "octaves" of 8 key-tiles

    from contextlib import ExitStack
    with tile.TileContext(nc) as tc, ExitStack() as ctx:
        singles = ctx.enter_context(tc.tile_pool(name="singles", bufs=1))
        hpool = ctx.enter_context(tc.tile_pool(name="hpool", bufs=12))
        ptpool = ctx.enter_context(tc.tile_pool(name="ptpool", bufs=4))
        rcpool = ctx.enter_context(tc.tile_pool(name="rcpool", bufs=2))
        # PSUM budget (8 banks): sc supers 2x[128,1024]=4, kv proj [128,1024]=2,
        # ot [65,512]=1, misc (q_ps / vtr / r_ps, strictly sequential)=1
        scps = ctx.enter_context(tc.tile_pool(name="scps", bufs=2, space="PSUM"))
        kvps = ctx.enter_context(tc.tile_pool(name="kvps", bufs=1, space="PSUM"))
        otps = ctx.enter_context(tc.tile_pool(name="otps", bufs=1, space="PSUM"))
        mips = ctx.enter_context(tc.tile_pool(name="mips", bufs=1, space="PSUM"))

        def _tctile(shape, dtype, _n=[0]):
            _n[0] += 1
            return singles.tile(shape, dtype, name=f"persist{_n[0]}", tag=f"persist{_n[0]}")

        # ---- persistent SBUF tensors ----
        # (weight DMA issued first -- needed by the very first matmul -- then
        # the first octave's h tiles; everything else goes on the gpsimd DMA
        # path so the sync queue reaches the h loads as early as possible)
        # weights first on the sync queue (the first matmuls need them), then
        # the h loads; late-needed constants ride the gpsimd software-DGE path
        wkv_sb = _tctile([128, 6, 128], BF16)
        wq_sb = _tctile([128, 6, D], BF16)
        bkv_sb = _tctile([128, 1], F32)
        bq_sb = _tctile([D, 1], F32)
        dead_sb = _tctile([128, 1], F32)
        tri_sb = _tctile([128, 256], BF16)
        ident = _tctile([128, 128], BF16)
        nc.sync.dma_start(out=wkv_sb, in_=wkv)
        nc.sync.dma_start(out=wq_sb, in_=wq)
        nc.gpsimd.dma_start(out=bkv_sb, in_=bkv)
        nc.gpsimd.dma_start(out=bq_sb, in_=bq)
        nc.gpsimd.dma_start(out=dead_sb, in_=dead)
        nc.gpsimd.dma_start(out=tri_sb, in_=tri2)
        nc.gpsimd.dma_start(out=ident, in_=ident_in)

        ones64 = _tctile([1, D], BF16)
        nc.vector.memset(ones64, 1.0)

        # HAM warm-up: the PE clock-gate opens (1.2 -> 2.4 GHz) only after
        # ~3.4us of sustained matmul activity.  The PE is idle anyway while
        # the first h tiles are in flight, so burn that time on dummy matmuls
        # to start the real projection at full clock.
        warm_sb = _tctile([128, 256], BF16)
        nc.vector.memset(warm_sb, 0.0)
        warm_ps = mips.tile([128, 256], F32, tag="mi")
        for i in range(10):
            nc.tensor.matmul(
                warm_ps, lhsT=warm_sb[:, 0:128], rhs=warm_sb,
                start=i == 0, stop=i == 9, skip_group_check=True,
            )

        KV = _tctile([128, S], BF16)                    # rows 0-63 K^T, 64-127 V^T
        QT = _tctile([D, N_SLOTS * CHUNK], BF16)        # Q^T/8 (+bias)
        VH = _tctile([128, (S // 128) * (D + 1)], BF16)  # V_hat tiles [128, 65] each
        out_sb = _tctile([D, N_SLOTS * CHUNK], F32)
        if ROW_TILE:
            KB2 = _tctile([128, S], BF16)               # rows 64-127: K^T copy
            QT2 = _tctile([128, N_SLOTS * CHUNK], BF16)  # rows 64-127: Q^T copy

        VH_r = VH.rearrange("p (i c) -> p i c", c=D + 1)
        nc.vector.memset(VH_r[:, :, D : D + 1], 1.0)   # ones column for denominators

        # ---- projection octave, as a stream of deferrable steps ----
        def emit_octave_dma(m):
            """Issue the six h-tile DMA loads for octave m (prefetch)."""
            hts = []
            for e in range(6):
                h = hpool.tile([128, 1024], BF16, tag="h")
                nc.sync.dma_start(
                    out=h, in_=ht[e * 128 : (e + 1) * 128, m * 1024 : (m + 1) * 1024]
                )
                hts.append(h)
            return hts

        def proj_steps(m, hts):
            """Yield matmul-group / epilogue / transpose steps for octave m,
            to be interleaved between attention supers."""
            kv_ps = kvps.tile([128, 1024], F32, tag="kv")
            q_ps = mips.tile([D, CHUNK], F32, tag="mi")

            def mm_group(e):
                st, sp = e == 0, e == 5
                for half in range(2):
                    nc.tensor.matmul(
                        kv_ps[:, half * 512 : (half + 1) * 512],
                        lhsT=wkv_sb[:, e, :],
                        rhs=hts[e][:, half * 512 : (half + 1) * 512],
                        start=st, stop=sp, skip_group_check=True,
                    )
                nc.tensor.matmul(
                    q_ps, lhsT=wq_sb[:, e, :],
                    rhs=hts[e][:, 512:1024],
                    start=st, stop=sp, skip_group_check=True,
                )

            for e in range(6):
                yield lambda e=e: mm_group(e)

            def epilogue():
                # on DVE (keep ACT free for exp)
                nc.vector.tensor_scalar_add(
                    KV[:, m * 1024 : (m + 1) * 1024], kv_ps, bkv_sb
                )
                nc.vector.tensor_scalar(
                    QT[:, m * CHUNK : (m + 1) * CHUNK], q_ps,
                    scalar1=0.125, scalar2=bq_sb,
                    op0=mybir.AluOpType.mult, op1=mybir.AluOpType.add,
                )
                if ROW_TILE:
                    # duplicate K^T / Q^T onto partitions 64-127 for row-group B
                    nc.sync.dma_start(
                        out=KB2[64:128, m * 1024 : (m + 1) * 1024],
                        in_=KV[0:64, m * 1024 : (m + 1) * 1024],
                    )
                    nc.sync.dma_start(
                        out=QT2[64:128, m * CHUNK : (m + 1) * CHUNK],
                        in_=QT[:, m * CHUNK : (m + 1) * CHUNK],
                    )

            yield epilogue

            # transpose [128, 128] KV blocks on the PE; streaming only columns
            # 64:128 of the identity keeps just the V rows of the transpose
            vtr_ps = mips.tile([128, 8 * D], BF16, tag="mi")
            for t in range(8):
                yield lambda t=t: nc.tensor.transpose(
                    vtr_ps[:, t * D : (t + 1) * D],
                    KV[:, m * 1024 + t * 128 : m * 1024 + (t + 1) * 128],
                    ident[:, D:128],
                )
            yield lambda: nc.vector.tensor_copy(
                VH_r[:, 8 * m : 8 * m + 8, 0:D],
                vtr_ps.rearrange("p (i c) -> p i c", c=D),
            )

        def run_all(steps):
            for s in steps:
                s()

        # ---- attention slot (optionally interleaving deferred steps) ----
        def emit_slot(j, deferred=()):
            deferred = iter(deferred)
            n_sup = 4 * (j + 1)
            ot_ps = otps.tile([D + 1, CHUNK], F32, tag="o")
            qt_j = QT[:, j * CHUNK : (j + 1) * CHUNK]
            if ROW_TILE:
                qt2_j = QT2[64:128, j * CHUNK : (j + 1) * CHUNK]
            for u in range(n_sup):
                sc_ps = scps.tile([128, 1024], F32, tag="s")
                kt0, kt1 = 2 * u, 2 * u + 1
                if ROW_TILE:
                    nc.tensor.matmul(
                        sc_ps[:, 0:512],
                        lhsT=KV[0:64, kt0 * 128 : (kt0 + 1) * 128],
                        rhs=qt_j, start=True, stop=True,
                    )
                    nc.tensor.matmul(
                        sc_ps[:, 512:1024],
                        lhsT=KB2[64:128, kt1 * 128 : (kt1 + 1) * 128],
                        rhs=qt2_j, start=True, stop=True,
                    )
                else:
                    for t, kt in ((0, kt0), (1, kt1)):
                        nc.tensor.matmul(
                            sc_ps[:, t * 512 : (t + 1) * 512],
                            lhsT=KV[0:64, kt * 128 : (kt + 1) * 128],
                            rhs=qt_j, start=True, stop=True,
                        )
                pt = ptpool.tile([128, 1024], BF16, tag="p")
                bias = dead_sb if u in (n_sup - 4, n_sup - 3) else 0.0
                if EXP_SPLIT:
                    for t in range(2):
                        nc.scalar.activation(
                            pt[:, t * 512 : (t + 1) * 512],
                            sc_ps[:, t * 512 : (t + 1) * 512],
                            mybir.ActivationFunctionType.Exp, bias=bias,
                        )
                else:
                    nc.scalar.activation(
                        pt, sc_ps, mybir.ActivationFunctionType.Exp, bias=bias
                    )
                if u == n_sup - 2:
                    # diag blocks (v=0,s=0) and (v=1,s=1): cols 0 and 640
                    nc.vector.tensor_mul(
                        _strided_cols(pt, 0, 640, 2, 128),
                        _strided_cols(pt, 0, 640, 2, 128),
                        tri_sb.rearrange("p (b c) -> p b c", c=128),
                    )
                    nc.vector.memset(pt[:, 512:640], 0.0)    # (v=1, s=0)
                if u == n_sup - 1:
                    # diag blocks (v=2,s=2) and (v=3,s=3): cols 256 and 896
                    nc.vector.tensor_mul(
                        _strided_cols(pt, 256, 640, 2, 128),
                        _strided_cols(pt, 256, 640, 2, 128),
                        tri_sb.rearrange("p (b c) -> p b c", c=128),
                    )
                    nc.vector.memset(pt[:, 0:256], 0.0)      # (v=2, s<2)
                    nc.vector.memset(pt[:, 512:896], 0.0)    # (v=3, s<3)
                for t in range(2):
                    ktile = 2 * u + t
                    nc.tensor.matmul(
                        ot_ps,
                        lhsT=VH_r[:, ktile, :],
                        rhs=pt[:, t * 512 : (t + 1) * 512],
                        start=(u == 0 and t == 0),
                        stop=(u == n_sup - 1 and t == 1),
                        skip_group_check=True,
                    )
                # inject a couple of deferred proj steps between supers to
                # keep the PE stream dense without starving the exp pipeline
                for _ in range(3):
                    step = next(deferred, None)
                    if step is None:
                        break
                    step()
            run_all(deferred)
            # normalize: out[:, q] = num[:, q] / den[q].  The numerator copy
            # runs on ScalarE (idle at slot end) in parallel with the DVE
            # reciprocal chain; both copies release ot_ps early so the next
            # slot's PV accumulation can claim the bank.
            # (den is copied to base partition 0 first: the custom-DVE
            # reciprocal mishandles nonzero base-partition sources)
            o_sl = out_sb[:, j * CHUNK : (j + 1) * CHUNK]
            den_sb = rcpool.tile([1, CHUNK], F32, tag="dn")
            nc.vector.tensor_copy(den_sb, ot_ps[D : D + 1, :])
            nc.scalar.copy(o_sl, ot_ps[0:D, :])
            rsb = rcpool.tile([1, CHUNK], F32, tag="rc")
            nc.vector.reciprocal_approx_fast(rsb, den_sb)
            rsb_bf = rcpool.tile([1, CHUNK], BF16, tag="rb")
            nc.vector.tensor_copy(rsb_bf, rsb)
            r_ps = mips.tile([D, CHUNK], F32, tag="mi")
            nc.tensor.matmul(r_ps, lhsT=ones64, rhs=rsb_bf, start=True, stop=True)
            nc.vector.tensor_mul(o_sl, o_sl, r_ps)
            nc.sync.dma_start(out=out[:, j * CHUNK : (j + 1) * CHUNK], in_=o_sl)

        # ---- interleaved emission: octave m's projection work (matmuls,
        # epilogue, transposes) is injected between the supers of slot m-1,
        # so the PE matmul stream never goes sparse and the exp pipeline
        # (ScalarE) never starves ----
        hts0 = emit_octave_dma(0)
        run_all(proj_steps(0, hts0))         # needed by slot 0 immediately
        hts1 = emit_octave_dma(1)
        emit_slot(0, proj_steps(1, hts1))
        hts2 = emit_octave_dma(2)
        emit_slot(1, proj_steps(2, hts2))
        hts3 = emit_octave_dma(3)
        emit_slot(2, proj_steps(3, hts3))
        emit_slot(3)
        if debug_dump:
            nc.sync.dma_start(out=kv_dbg, in_=KV)
            nc.sync.dma_start(out=qt_dbg, in_=QT)
            nc.sync.dma_start(out=vh_dbg, in_=VH)

    nc.finalize()
    return nc


_NC_CACHE = []


def _get_nc():
    if not _NC_CACHE:
        _NC_CACHE.append(build())
    return _NC_CACHE[0]


def make_in_maps(hidden_state, Wq, bq, Wk, bk, Wv, bv):
    hidden_state = np.asarray(hidden_state, dtype=np.float32)
    tri = np.triu(np.ones((128, 128), dtype=np.float32))  # keep iff q_free >= k_part
    tri2_np = np.concatenate([tri, tri], axis=1).astype(ml_dtypes.bfloat16)

    def chunked(w):  # [768, X] -> [128, 6, X]
        x = np.asarray(w, np.float32)
        return np.ascontiguousarray(
            x.reshape(6, 128, x.shape[1]).transpose(1, 0, 2)
        ).astype(ml_dtypes.bfloat16)

    wkv_np = chunked(np.concatenate([np.asarray(Wk, np.float32),
                                     np.asarray(Wv, np.float32)], axis=1))
    wq_np = chunked(np.asarray(Wq, np.float32))
    bkv_np = np.ascontiguousarray(
        np.concatenate([np.asarray(bk, np.float32), np.asarray(bv, np.float32)])[:, None]
    )
    base_w = {
        "wkv": wkv_np,
        "wq": wq_np,
        "bkv": bkv_np,
        "bq": np.ascontiguousarray((np.asarray(bq, np.float32) * 0.125)[:, None]),
        "tri2": tri2_np,
        "ident": np.eye(128, dtype=ml_dtypes.bfloat16),
    }
    in_maps = []
    for c in range(N_CORES):
        b, p = c // 2, c % 2
        hT = np.ascontiguousarray(hidden_state[b].T)  # [E, S]
        if p == 1:
            hT = np.ascontiguousarray(
                hT.reshape(E, S // 1024, 2, 512)[:, :, ::-1, :].reshape(E, S)
            )
        dead_np = np.full((128, 1), DEAD_BIAS if p == 1 else 0.0, dtype=np.float32)
        in_maps.append(
            {"ht": hT.astype(ml_dtypes.bfloat16), "dead": dead_np, **base_w}
        )
    return in_maps


def gather_output(results):
    OUT = np.empty((B, S, D), dtype=np.float32)
    for c in range(N_CORES):
        b, p = c // 2, c % 2
        o = results[c]["out"]  # [64, 2048]
        for j in range(N_SLOTS):
            chunk = 2 * j + 1 - p
            OUT[b, chunk * CHUNK : (chunk + 1) * CHUNK, :] = o[
                :, j * CHUNK : (j + 1) * CHUNK
            ].T
    return OUT


def run_cores(in_maps, **kwargs):
    nc = _get_nc()
    return bass_utils.run_bass_kernel_spmd(
        nc, in_maps, core_ids=list(range(N_CORES)), **kwargs
    )


def kernel(hidden_state, Wq, bq, Wk, bk, Wv, bv):
    in_maps = make_in_maps(hidden_state, Wq, bq, Wk, bk, Wv, bv)
    res = run_cores(in_maps)
    return gather_output(res.results)


# revision 57
# speedup vs baseline: 1.0461x; 1.0461x over previous
"""Single-head causal attention (B=4, S=4096, E=768, D=64) on 8 TRN2 NeuronCores.

Sharding: data-parallel over (batch, query-half): core c -> batch c//2, half c%2.
Each core receives its batch's hidden state pre-transposed to [E, S] in bf16
(host-side layout + dtype choice) and computes attention output for 2048 queries.

Per-core schedule (SPMD-uniform): queries are processed in 4 slots of 512.
Slot j covers keys [0, 1024*(j+1)).  For half p=0 the core owns the upper
512-query chunk of each 1024-block, for p=1 the lower one; the host swaps the
two 512-column halves of each 1024-key block of h^T for p=1 so both halves run
the *same* program:
  - keys [0, 1024j)            : fully unmasked
  - keys [1024j, 1024j+512)    : "dead zone" -- past for p=0 (keep), future for
                                 p=1 (killed via per-core exp bias of -80)
  - keys [1024j+512, 1024(j+1)): the core's own 512 queries -> block-triangular
                                 (0/1 multiplicative mask + memsets on exp out)

Projection: one packed [Wk|Wv] matmul per (octave, e-chunk) produces K^T stacked
on V^T in a pair of [128, 512] PSUM banks; DVE applies the stacked bias and
writes the KV tensor (K^T rows 0-63, V^T rows 64-127). V tiles for the PV
matmul come from PE transposes of a base-partition-0 staging copy; the
transposes are spread across the *next* slot's supers so the PE matmul stream
stays dense (HAM clock-gate stays warm). Q (own half only) gets bias + 1/8
scale on DVE.

Attention per slot: supers of 2 key-tiles. The two score matmuls of a super run
*concurrently* in different PE row-groups (contract dim is only 64): tile A
contracts array rows 0-63 (K^T / Q^T at partitions 0-63), tile B rows 64-127
using copies of K^T / Q^T DMA-duplicated to partitions 64-127. One Exp ACTIVATE
per super (per-core dead bias folded in); tri-mask on DVE; PV accumulates
[V|1]^T @ P so row 64 collects softmax denominators. Normalization: DVE
reciprocal_approx_fast on the denominator row, bf16 ones-broadcast matmul, DVE
multiply. Emission order interleaves proj octaves with attention slots.
"""

import numpy as np
import ml_dtypes

import concourse.bass as bass
import concourse.tile as tile
from concourse import bacc, mybir
from concourse import bass_utils
from concourse.masks import make_identity

B, S, E, D = 4, 4096, 768, 64
N_CORES = 8
CHUNK = 512            # queries per slot
N_SLOTS = 4            # slots per core (4 * 512 = 2048 queries)
F32 = mybir.dt.float32
BF16 = mybir.dt.bfloat16
DEAD_BIAS = -80.0
ROW_TILE = True        # run each super's two score matmuls in parallel row-groups
EXP_SPLIT = False      # two [128,512] Exp ACTIVATEs per super instead of one [128,1024]


def _strided_cols(t: bass.AP, start: int, stride: int, count: int, width: int) -> bass.AP:
    """AP selecting `count` column-blocks of `width` at `start`, `start+stride`, ..."""
    base = t[:, start : start + 1]
    return bass.AP(
        tensor=base.tensor,
        offset=base.offset,
        ap=[base.ap[0], [stride, count], [1, width]],
    )


def build(debug_dump=False):
    nc = bacc.Bacc("TRN2", target_bir_lowering=False, debug=False, num_devices=N_CORES)

    ht = nc.dram_tensor("ht", [E, S], BF16, kind="ExternalInput").ap()
    wkv = nc.dram_tensor("wkv", [128, 6, 128], BF16, kind="ExternalInput").ap()
    wq = nc.dram_tensor("wq", [128, 6, D], BF16, kind="ExternalInput").ap()
    bkv = nc.dram_tensor("bkv", [128, 1], F32, kind="ExternalInput").ap()
    bq = nc.dram_tensor("bq", [D, 1], F32, kind="ExternalInput").ap()
    dead = nc.dram_tensor("dead", [128, 1], F32, kind="ExternalInput").ap()
    tri2 = nc.dram_tensor("tri2", [128, 256], BF16, kind="ExternalInput").ap()
    ident_in = nc.dram_tensor("ident", [128, 128], BF16, kind="ExternalInput").ap()
    out = nc.dram_tensor("out", [D, N_SLOTS * CHUNK], F32, kind="ExternalOutput").ap()
    if debug_dump:
        kv_dbg = nc.dram_tensor("kv_dbg", [128, S], BF16, kind="ExternalOutput").ap()
        qt_dbg = nc.dram_tensor("qt_dbg", [D, N_SLOTS * CHUNK], BF16, kind="ExternalOutput").ap()
        vh_dbg = nc.dram_tensor("vh_dbg", [128, (S // 128) * (D + 1)], BF16, kind="ExternalOutput").ap()

    n_oct = S // 1024            # 4 "octaves" of 8 key-tiles

    from contextlib import ExitStack
    with tile.TileContext(nc) as tc, ExitStack() as ctx:
        singles = ctx.enter_context(tc.tile_pool(name="singles", bufs=1))
        hpool = ctx.enter_context(tc.tile_pool(name="hpool", bufs=12))
        ptpool = ctx.enter_context(tc.tile_pool(name="ptpool", bufs=4))
        rcpool = ctx.enter_context(tc.tile_pool(name="rcpool", bufs=2))
        # PSUM budget (8 banks): sc supers 2x[128,1024]=4, kv proj [128,1024]=2,
        # ot [65,512]=1, misc (q_ps / vtr / r_ps, strictly sequential)=1
        scps = ctx.enter_context(tc.tile_pool(name="scps", bufs=2, space="PSUM"))
        kvps = ctx.enter_context(tc.tile_pool(name="kvps", bufs=1, space="PSUM"))
        otps = ctx.enter_context(tc.tile_pool(name="otps", bufs=1, space="PSUM"))
        mips = ctx.enter_context(tc.tile_pool(name="mips", bufs=1, space="PSUM"))

        def _tctile(shape, dtype, _n=[0]):
            _n[0] += 1
            return singles.tile(shape, dtype, name=f"persist{_n[0]}", tag=f"persist{_n[0]}")

        # ---- persistent SBUF tensors ----
        # (weight DMA issued first -- needed by the very first matmul -- then
        # the first octave's h tiles; everything else goes on the gpsimd DMA
        # path so the sync queue reaches the h loads as early as possible)
        # weights first on the sync queue (the first matmuls need them), then
        # the h loads; late-needed constants ride the gpsimd software-DGE path
        wkv_sb = _tctile([128, 6, 128], BF16)
        wq_sb = _tctile([128, 6, D], BF16)
        bkv_sb = _tctile([128, 1], F32)
        bq_sb = _tctile([D, 1], F32)
        dead_sb = _tctile([128, 1], F32)
        tri_sb = _tctile([128, 256], BF16)
        ident = _tctile([128, 128], BF16)
        nc.sync.dma_start(out=wkv_sb, in_=wkv)
        nc.sync.dma_start(out=wq_sb, in_=wq)
        nc.gpsimd.dma_start(out=bkv_sb, in_=bkv)
        nc.gpsimd.dma_start(out=bq_sb, in_=bq)
        nc.gpsimd.dma_start(out=dead_sb, in_=dead)
        nc.gpsimd.dma_start(out=tri_sb, in_=tri2)
        nc.gpsimd.dma_start(out=ident, in_=ident_in)

        ones64 = _tctile([1, D], BF16)
        nc.vector.memset(ones64, 1.0)

        # HAM warm-up: the PE clock-gate opens (1.2 -> 2.4 GHz) only after
        # ~3.4us of sustained matmul activity.  The PE is idle anyway while
        # the first h tiles are in flight, so burn that time on dummy matmuls
        # to start the real projection at full clock.
        warm_sb = _tctile([128, 128], BF16)
        nc.vector.memset(warm_sb, 0.0)
        warm_ps = mips.tile([128, 128], F32, tag="mi")
        for i in range(22):
            nc.tensor.matmul(
                warm_ps, lhsT=warm_sb, rhs=warm_sb,
                start=i == 0, stop=i == 21, skip_group_check=True,
            )

        KV = _tctile([128, S], BF16)                    # rows 0-63 K^T, 64-127 V^T
        QT = _tctile([D, N_SLOTS * CHUNK], BF16)        # Q^T/8 (+bias)
        VH = _tctile([128, (S // 128) * (D + 1)], BF16)  # V_hat tiles [128, 65] each
        out_sb = _tctile([D, N_SLOTS * CHUNK], F32)
        if ROW_TILE:
            KB2 = _tctile([128, S], BF16)               # rows 64-127: K^T copy
            QT2 = _tctile([128, N_SLOTS * CHUNK], BF16)  # rows 64-127: Q^T copy

        VH_r = VH.rearrange("p (i c) -> p i c", c=D + 1)
        nc.vector.memset(VH_r[:, :, D : D + 1], 1.0)   # ones column for denominators

        # ---- projection octave, as a stream of deferrable steps ----
        def emit_octave_dma(m):
            """Issue the six h-tile DMA loads for octave m (prefetch)."""
            hts = []
            for e in range(6):
                h = hpool.tile([128, 1024], BF16, tag="h")
                nc.sync.dma_start(
                    out=h, in_=ht[e * 128 : (e + 1) * 128, m * 1024 : (m + 1) * 1024]
                )
                hts.append(h)
            return hts

        def proj_steps(m, hts):
            """Yield matmul-group / epilogue / transpose steps for octave m,
            to be interleaved between attention supers."""
            kv_ps = kvps.tile([128, 1024], F32, tag="kv")
            q_ps = mips.tile([D, CHUNK], F32, tag="mi")

            def mm_group(e):
                st, sp = e == 0, e == 5
                for half in range(2):
                    nc.tensor.matmul(
                        kv_ps[:, half * 512 : (half + 1) * 512],
                        lhsT=wkv_sb[:, e, :],
                        rhs=hts[e][:, half * 512 : (half + 1) * 512],
                        start=st, stop=sp, skip_group_check=True,
                    )
                nc.tensor.matmul(
                    q_ps, lhsT=wq_sb[:, e, :],
                    rhs=hts[e][:, 512:1024],
                    start=st, stop=sp, skip_group_check=True,
                )

            for e in range(6):
                yield lambda e=e: mm_group(e)

            def epilogue():
                # on DVE (keep ACT free for exp)
                nc.vector.tensor_scalar_add(
                    KV[:, m * 1024 : (m + 1) * 1024], kv_ps, bkv_sb
                )
                nc.vector.tensor_scalar(
                    QT[:, m * CHUNK : (m + 1) * CHUNK], q_ps,
                    scalar1=0.125, scalar2=bq_sb,
                    op0=mybir.AluOpType.mult, op1=mybir.AluOpType.add,
                )
                if ROW_TILE:
                    # duplicate K^T / Q^T onto partitions 64-127 for row-group B
                    nc.sync.dma_start(
                        out=KB2[64:128, m * 1024 : (m + 1) * 1024],
                        in_=KV[0:64, m * 1024 : (m + 1) * 1024],
                    )
                    nc.sync.dma_start(
                        out=QT2[64:128, m * CHUNK : (m + 1) * CHUNK],
                        in_=QT[:, m * CHUNK : (m + 1) * CHUNK],
                    )

            yield epilogue

            # transpose [128, 128] KV blocks on the PE; streaming only columns
            # 64:128 of the identity keeps just the V rows of the transpose
            vtr_ps = mips.tile([128, 8 * D], BF16, tag="mi")
            for t in range(8):
                yield lambda t=t: nc.tensor.transpose(
                    vtr_ps[:, t * D : (t + 1) * D],
                    KV[:, m * 1024 + t * 128 : m * 1024 + (t + 1) * 128],
                    ident[:, D:128],
                )
            yield lambda: nc.vector.tensor_copy(
                VH_r[:, 8 * m : 8 * m + 8, 0:D],
                vtr_ps.rearrange("p (i c) -> p i c", c=D),
            )

        def run_all(steps):
            for s in steps:
                s()

        # ---- attention slot (optionally interleaving deferred steps) ----
        def emit_slot(j, deferred=()):
            deferred = iter(deferred)
            n_sup = 4 * (j + 1)
            ot_ps = otps.tile([D + 1, CHUNK], F32, tag="o")
            qt_j = QT[:, j * CHUNK : (j + 1) * CHUNK]
            if ROW_TILE:
                qt2_j = QT2[64:128, j * CHUNK : (j + 1) * CHUNK]
            for u in range(n_sup):
                sc_ps = scps.tile([128, 1024], F32, tag="s")
                kt0, kt1 = 2 * u, 2 * u + 1
                if ROW_TILE:
                    nc.tensor.matmul(
                        sc_ps[:, 0:512],
                        lhsT=KV[0:64, kt0 * 128 : (kt0 + 1) * 128],
                        rhs=qt_j, start=True, stop=True,
                    )
                    nc.tensor.matmul(
                        sc_ps[:, 512:1024],
                        lhsT=KB2[64:128, kt1 * 128 : (kt1 + 1) * 128],
                        rhs=qt2_j, start=True, stop=True,
                    )
                else:
                    for t, kt in ((0, kt0), (1, kt1)):
                        nc.tensor.matmul(
                            sc_ps[:, t * 512 : (t + 1) * 512],
                            lhsT=KV[0:64, kt * 128 : (kt + 1) * 128],
                            rhs=qt_j, start=True, stop=True,
                        )
                pt = ptpool.tile([128, 1024], BF16, tag="p")
                bias = dead_sb if u in (n_sup - 4, n_sup - 3) else 0.0
                if EXP_SPLIT:
                    for t in range(2):
                        nc.scalar.activation(
                            pt[:, t * 512 : (t + 1) * 512],
                            sc_ps[:, t * 512 : (t + 1) * 512],
                            mybir.ActivationFunctionType.Exp, bias=bias,
                        )
                else:
                    nc.scalar.activation(
                        pt, sc_ps, mybir.ActivationFunctionType.Exp, bias=bias
                    )
                if u == n_sup - 2:
                    # diag blocks (v=0,s=0) and (v=1,s=1): cols 0 and 640
                    nc.vector.tensor_mul(
                        _strided_cols(pt, 0, 640, 2, 128),
                        _strided_cols(pt, 0, 640, 2, 128),
                        tri_sb.rearrange("p (b c) -> p b c", c=128),
                    )
                    nc.vector.memset(pt[:, 512:640], 0.0)    # (v=1, s=0)
                if u == n_sup - 1:
                    # diag blocks (v=2,s=2) and (v=3,s=3): cols 256 and 896
                    nc.vector.tensor_mul(
                        _strided_cols(pt, 256, 640, 2, 128),
                        _strided_cols(pt, 256, 640, 2, 128),
                        tri_sb.rearrange("p (b c) -> p b c", c=128),
                    )
                    nc.vector.memset(pt[:, 0:256], 0.0)      # (v=2, s<2)
                    nc.vector.memset(pt[:, 512:896], 0.0)    # (v=3, s<3)
                for t in range(2):
                    ktile = 2 * u + t
                    nc.tensor.matmul(
                        ot_ps,
                        lhsT=VH_r[:, ktile, :],
                        rhs=pt[:, t * 512 : (t + 1) * 512],
                        start=(u == 0 and t == 0),
                        stop=(u == n_sup - 1 and t == 1),
                        skip_group_check=True,
                    )
                # inject a couple of deferred proj steps between supers to
                # keep the PE stream dense without starving the exp pipeline
                for _ in range(3):
                    step = next(deferred, None)
                    if step is None:
                        break
                    step()
            run_all(deferred)
            # normalize: out[:, q] = num[:, q] / den[q].  The numerator copy
            # runs on ScalarE (idle at slot end) in parallel with the DVE
            # reciprocal chain; both copies release ot_ps early so the next
            # slot's PV accumulation can claim the bank.
            # (den is copied to base partition 0 first: the custom-DVE
            # reciprocal mishandles nonzero base-partition sources)
            o_sl = out_sb[:, j * CHUNK : (j + 1) * CHUNK]
            den_sb = rcpool.tile([1, CHUNK], F32, tag="dn")
            nc.vector.tensor_copy(den_sb, ot_ps[D : D + 1, :])
            nc.scalar.copy(o_sl, ot_ps[0:D, :])
            rsb = rcpool.tile([1, CHUNK], F32, tag="rc")
            nc.vector.reciprocal_approx_fast(rsb, den_sb)
            rsb_bf = rcpool.tile([1, CHUNK], BF16, tag="rb")
            nc.vector.tensor_copy(rsb_bf, rsb)
            r_ps = mips.tile([D, CHUNK], F32, tag="mi")
            nc.tensor.matmul(r_ps, lhsT=ones64, rhs=rsb_bf, start=True, stop=True)
            nc.vector.tensor_mul(o_sl, o_sl, r_ps)
            nc.sync.dma_start(out=out[:, j * CHUNK : (j + 1) * CHUNK], in_=o_sl)

        # ---- interleaved emission: octave m's projection work (matmuls,
        # epilogue, transposes) is injected between the supers of slot m-1,
        # so the PE matmul stream never goes sparse and the exp pipeline
        # (ScalarE) never starves ----
        hts0 = emit_octave_dma(0)
        run_all(proj_steps(0, hts0))         # needed by slot 0 immediately
        hts1 = emit_octave_dma(1)
        emit_slot(0, proj_steps(1, hts1))
        hts2 = emit_octave_dma(2)
        emit_slot(1, proj_steps(2, hts2))
        hts3 = emit_octave_dma(3)
        emit_slot(2, proj_steps(3, hts3))
        emit_slot(3)
        if debug_dump:
            nc.sync.dma_start(out=kv_dbg, in_=KV)
            nc.sync.dma_start(out=qt_dbg, in_=QT)
            nc.sync.dma_start(out=vh_dbg, in_=VH)

    nc.finalize()
    return nc


_NC_CACHE = []


def _get_nc():
    if not _NC_CACHE:
        _NC_CACHE.append(build())
    return _NC_CACHE[0]


def make_in_maps(hidden_state, Wq, bq, Wk, bk, Wv, bv):
    hidden_state = np.asarray(hidden_state, dtype=np.float32)
    tri = np.triu(np.ones((128, 128), dtype=np.float32))  # keep iff q_free >= k_part
    tri2_np = np.concatenate([tri, tri], axis=1).astype(ml_dtypes.bfloat16)

    def chunked(w):  # [768, X] -> [128, 6, X]
        x = np.asarray(w, np.float32)
        return np.ascontiguousarray(
            x.reshape(6, 128, x.shape[1]).transpose(1, 0, 2)
        ).astype(ml_dtypes.bfloat16)

    wkv_np = chunked(np.concatenate([np.asarray(Wk, np.float32),
                                     np.asarray(Wv, np.float32)], axis=1))
    wq_np = chunked(np.asarray(Wq, np.float32))
    bkv_np = np.ascontiguousarray(
        np.concatenate([np.asarray(bk, np.float32), np.asarray(bv, np.float32)])[:, None]
    )
    base_w = {
        "wkv": wkv_np,
        "wq": wq_np,
        "bkv": bkv_np,
        "bq": np.ascontiguousarray((np.asarray(bq, np.float32) * 0.125)[:, None]),
        "tri2": tri2_np,
        "ident": np.eye(128, dtype=ml_dtypes.bfloat16),
    }
    in_maps = []
    for c in range(N_CORES):
        b, p = c // 2, c % 2
        hT = np.ascontiguousarray(hidden_state[b].T)  # [E, S]
        if p == 1:
            hT = np.ascontiguousarray(
                hT.reshape(E, S // 1024, 2, 512)[:, :, ::-1, :].reshape(E, S)
            )
        dead_np = np.full((128, 1), DEAD_BIAS if p == 1 else 0.0, dtype=np.float32)
        in_maps.append(
            {"ht": hT.astype(ml_dtypes.bfloat16), "dead": dead_np, **base_w}
        )
    return in_maps


def gather_output(results):
    OUT = np.empty((B, S, D), dtype=np.float32)
    for c in range(N_CORES):
        b, p = c // 2, c % 2
        o = results[c]["out"]  # [64, 2048]
        for j in range(N_SLOTS):
            chunk = 2 * j + 1 - p
            OUT[b, chunk * CHUNK : (chunk + 1) * CHUNK, :] = o[
                :, j * CHUNK : (j + 1) * CHUNK
            ].T
    return OUT


def run_cores(in_maps, **kwargs):
    nc = _get_nc()
    return bass_utils.run_bass_kernel_spmd(
        nc, in_maps, core_ids=list(range(N_CORES)), **kwargs
    )


def kernel(hidden_state, Wq, bq, Wk, bk, Wv, bv):
    in_maps = make_in_maps(hidden_state, Wq, bq, Wk, bk, Wv, bv)
    res = run_cores(in_maps)
    return gather_output(res.results)
